# revision 12
# baseline (speedup 1.0000x reference)
"""Bass/Trainium2 kernel for nn_F_Loss_65446711656630.

Strategy (data-parallel over N, 8 cores):
  - Host: per core slice 8192 rows (64 chunks of 128 rows), lay them out
    rows-on-partitions in 11 DMA tiles with a mixed-precision schedule:
    5 "mega" tiles of [128 x 4096] fp8e4m3 (8 chunks each; 4 KB per
    partition per tile keeps DMA descriptors at full efficiency) inter-
    leaved with 6 tiles of [128 x 2048] fp16 (4 chunks each).  ~6.2 MiB
    per core vs 8.3 for all-fp16.  Plus per-chunk one-hot class matrices
    W [128 rows, 64 chunks * 16 classes] in fp16 and fp8.  No sorting, no
    boundary fixups: the one-hot IS the segment assignment.
  - Device: segment-sum as matmul on the TensorEngine: for each 128-row
    chunk k,  psum[16, 512] += W_k^T @ chunk  gives the per-class sums
    directly; a second accumulator takes W_k^T @ square(chunk).  Squares
    (fp16 out, for precision) are split ACT (4 fp8 megas, 1x) / DVE
    (1 mega + all fp16 tiles, 2x for fp16) interleaved with arrival order
    so both engines run concurrently under the DMA stream rate.  Four
    PSUM accumulators (h/sq x even/odd chunks) live in four banks at
    column-group offsets 0/32/64/96 so consecutive matmuls target
    different PE column groups and overlap.
  - Host: add the 4 accumulators (fp64), then the tiny O(C^2 D) pairwise
    betainc/top-k stage in f32 jax on CPU (mirroring the reference's
    numerics exactly).
"""

import numpy as np

C = 16
D = 512
N = 65536
NCORES = 8
ROWS = N // NCORES          # 8192 rows per core
P = 128                     # SBUF partitions / rows per chunk
CHUNKS = ROWS // P          # 64 chunks per core
XMIN, XMAX = 1e-37, 1.0 - 1e-5

# tile schedule in arrival order: (kind, nchunks, square engine)
#   "m" = fp8e4m3 mega tile, "f" = fp16 tile; chunks assigned sequentially
TILES = [
    ("m", 8, "act"), ("f", 4, "dve"), ("m", 8, "act"), ("f", 4, "dve"),
    ("m", 8, "act"), ("f", 4, "dve"), ("m", 8, "act"), ("f", 4, "dve"),
    ("m", 8, "dve"), ("f", 4, "dve"), ("f", 4, "dve"),
]
assert sum(t[1] for t in TILES) == CHUNKS
NM = sum(1 for t in TILES if t[0] == "m")   # 5 fp8 mega tiles
NF = sum(1 for t in TILES if t[0] == "f")   # 6 fp16 tiles
MCOLS = 8 * D                                # 4096
FCOLS = 4 * D                                # 2048

_NC_CACHE = {}


def _tile_chunk_ranges():
    out, base = [], 0
    for kind, nch, eng in TILES:
        out.append((kind, base, base + nch, eng))
        base += nch
    return out


def _build_nc():
    """Per-core SPMD program.

    Inputs:  "htm" [5, 128, 4096] fp8e4  (mega tiles, arrival-order slot;
                                          col (c*512+f) of slot s =
                                          hidden[(chunk_base+c)*128+r, f])
             "htf" [6, 128, 2048] fp16   (fp16 tiles)
             "w16"/"w8" [128, 1024]      (w[r, k*16+q] = 1 iff row r of
                                          chunk k has class q)
    Output:  "ob" [128, 512] f32         (partitions 32g..32g+16 = group g:
                                          0 h-sums even chunks, 1 h-sums
                                          odd, 2 sq-sums even, 3 sq odd)
    """
    import concourse.tile as tile
    from concourse import bacc, mybir

    f32 = mybir.dt.float32
    f16 = mybir.dt.float16
    f8 = mybir.dt.float8e4

    nc = bacc.Bacc("TRN2", target_bir_lowering=False, debug=False,
                   num_devices=NCORES)
    htm = nc.declare_dram_parameter("htm", [NM, P, MCOLS], f8, isOutput=False)
    htf = nc.declare_dram_parameter("htf", [NF, P, FCOLS], f16,
                                    isOutput=False)
    w16 = nc.declare_dram_parameter("w16", [P, CHUNKS * C], f16,
                                    isOutput=False)
    w8 = nc.declare_dram_parameter("w8", [P, CHUNKS * C], f8, isOutput=False)
    ob = nc.declare_dram_parameter("ob", [P, D], f32, isOutput=True)

    with tile.TileContext(nc) as tc:
        with (
            tc.tile_pool(name="pcm", bufs=3) as pool_m,
            tc.tile_pool(name="pcf", bufs=3) as pool_f,
            tc.tile_pool(name="sqm", bufs=2) as sqm_pool,
            tc.tile_pool(name="sqf", bufs=3) as sqf_pool,
            tc.tile_pool(name="wp", bufs=1) as w_pool,
            tc.psum_pool(name="ps", bufs=1) as psum_pool,
        ):
            # hoist the ACT table load to the head of the program
            dummy = w_pool.tile([P, 8], f16, tag="dummy")
            nc.gpsimd.memset(dummy[:], 0)
            nc.scalar.square(dummy[:], dummy[:])

            wsb16 = w_pool.tile([P, CHUNKS * C], f16, tag="wsb16")
            nc.sync.dma_start(wsb16[:], w16[:])
            wsb8 = w_pool.tile([P, CHUNKS * C], f8, tag="wsb8")
            nc.sync.dma_start(wsb8[:], w8[:])

            banks = [psum_pool.tile([P, D], f32, name=f"acc{g}",
                                    tag=f"acc{g}")
                     for g in range(4)]
            accs = [banks[g][32 * g:32 * g + C, :] for g in range(4)]
            started = [False] * 4
            n_issued = [0] * 4

            def seg_mm(g, k, wsb, rhs):
                first = not started[g]
                started[g] = True
                n_issued[g] += 1
                last = n_issued[g] == CHUNKS // 2
                nc.tensor.matmul(
                    accs[g], wsb[:, k * C:(k + 1) * C], rhs,
                    start=first, stop=last, tile_position=(0, 32 * g))

            sq_done = []   # (sq tile, chunk base, nchunks)

            def issue_sq_mms():
                while sq_done:
                    sq, base, nch = sq_done.pop(0)
                    for c in range(nch):
                        k = base + c
                        seg_mm(2 + (k % 2), k, wsb16,
                               sq[:, c * D:(c + 1) * D])

            mslot = fslot = 0
            for i, (kind, base, end, eng) in enumerate(_tile_chunk_ranges()):
                nch = end - base
                if kind == "m":
                    t = pool_m.tile([P, MCOLS], f8, name=f"tm_{i}", tag="tm")
                    nc.gpsimd.dma_start(t[:], htm[mslot])
                    mslot += 1
                    hw = wsb8
                else:
                    t = pool_f.tile([P, FCOLS], f16, name=f"tf_{i}", tag="tf")
                    nc.sync.dma_start(t[:], htf[fslot])
                    fslot += 1
                    hw = wsb16

                spool = sqm_pool if kind == "m" else sqf_pool
                sq = spool.tile([P, nch * D], f16, name=f"sq_{i}",
                                tag="sqm" if kind == "m" else "sqf")
                if eng == "act":
                    nc.scalar.square(sq[:], t[:])
                else:
                    nc.vector.tensor_mul(sq[:], t[:], t[:])

                # h matmuls for this tile; sq matmuls for finished tiles
                for c in range(nch):
                    k = base + c
                    seg_mm(k % 2, k, hw, t[:, c * D:(c + 1) * D])
                issue_sq_mms()
                sq_done.append((sq, base, nch))
            issue_sq_mms()

            ob_sb = w_pool.tile([P, D], f32, tag="ob_sb")
            for g in range(4):
                if g % 2 == 0:
                    nc.vector.tensor_copy(ob_sb[32 * g:32 * g + C, :], accs[g])
                else:
                    nc.scalar.copy(ob_sb[32 * g:32 * g + C, :], accs[g])
            # h-sum half can ship while the sq copies still run
            nc.sync.dma_start(ob[0:64], ob_sb[0:64])
            nc.sync.dma_start(ob[64:128], ob_sb[64:128])
    nc.compile()
    return nc


def _get_nc():
    if "nc" not in _NC_CACHE:
        _NC_CACHE["nc"] = _build_nc()
    return _NC_CACHE["nc"]


def _prep_core(h_k, ids_k):
    import ml_dtypes

    ch = h_k.reshape(CHUNKS, P, D)
    Tm = np.empty((NM, P, MCOLS), dtype=ml_dtypes.float8_e4m3)
    Tf = np.empty((NF, P, FCOLS), dtype=np.float16)
    mslot = fslot = 0
    for kind, base, end, _ in _tile_chunk_ranges():
        blk = ch[base:end].transpose(1, 0, 2).reshape(P, (end - base) * D)
        if kind == "m":
            Tm[mslot] = blk.astype(ml_dtypes.float8_e4m3)
            mslot += 1
        else:
            Tf[fslot] = blk.astype(np.float16)
            fslot += 1

    ids2 = ids_k.reshape(CHUNKS, P)
    W3 = np.zeros((P, CHUNKS, C), dtype=np.float16)
    k_idx = np.broadcast_to(np.arange(CHUNKS)[:, None], (CHUNKS, P))
    r_idx = np.broadcast_to(np.arange(P)[None, :], (CHUNKS, P))
    W3[r_idx, k_idx, ids2] = 1.0
    W16 = W3.reshape(P, CHUNKS * C)
    return Tm, Tf, W16, W16.astype(ml_dtypes.float8_e4m3)


def _device_stats(hidden, ids, **run_kwargs):
    """Returns (sums[C,D], sumsq[C,D]) float64, plus the raw run result."""
    from concourse import bass_utils

    nc = _get_nc()

    in_maps = []
    for k in range(NCORES):
        rows = slice(k * ROWS, (k + 1) * ROWS)
        Tm, Tf, W16, W8 = _prep_core(hidden[rows], ids[rows])
        in_maps.append({"htm": Tm, "htf": Tf, "w16": W16, "w8": W8})

    res = bass_utils.run_bass_kernel_spmd(nc, in_maps, list(range(NCORES)),
                                          **run_kwargs)

    sums = np.zeros((C, D), dtype=np.float64)
    sumsq = np.zeros((C, D), dtype=np.float64)
    for k in range(NCORES):
        ob = res.results[k]["ob"].astype(np.float64)
        sums += ob[0:C] + ob[32:32 + C]
        sumsq += ob[64:64 + C] + ob[96:96 + C]
    return sums, sumsq, res


def _pairwise_loss(counts, sums, sumsq, d):
    """The tiny O(C^2 D) stage on host CPU.

    Runs in float32 with the same jax ops as the reference: at these extreme
    betainc parameters (b ~ 8190, x ~ 1e-5) jax's f32 betainc differs from
    the true (f64) value by ~1e-3, so matching the reference requires
    replicating its f32 numerics, not improving on them.
    """
    import jax
    import jax.numpy as jnp

    cpu = jax.devices("cpu")[0]
    with jax.default_device(cpu):
        counts64 = counts.astype(np.float64)
        means64 = sums / counts64[:, None]
        withins64 = sumsq - counts64[:, None] * means64**2
        counts = jnp.asarray(counts64, jnp.float32)               # [C]
        means = jnp.asarray(means64, jnp.float32)                 # [C, D]
        withins = jnp.asarray(withins64, jnp.float32)             # [C, D]
        half_diff = (means[:, None, :] - means[None, :, :]) * 0.5
        pair_counts = counts[:, None] + counts[None, :]
        pair_between = half_diff * half_diff * pair_counts[:, :, None]
        pair_within = withins[:, None, :] + withins[None, :, :]
        d2 = pair_counts - 2.0
        d2 = jnp.where(d2 == 0.0, 1e-5, d2)
        x = pair_between / (pair_between + pair_within)
        x = jnp.clip(x, XMIN, XMAX)
        a = jnp.full_like(x, 0.5)
        b = jnp.broadcast_to((d2 * 0.5)[:, :, None], x.shape)
        xbetainc = jax.scipy.special.betainc(a, b, x)             # [C, C, D]
        top_k, _ = jax.lax.top_k(xbetainc, int(d))                # [C, C, d]
        per_pair = jnp.sum(jnp.log(top_k), axis=-1)               # [C, C]
        mask = jnp.triu(jnp.ones((C, C), dtype=bool), k=1)
        total = jnp.sum(jnp.where(mask, per_pair, jnp.zeros_like(per_pair)))
        return float(-total)


def kernel(hidden, batch_ids, d):
    hidden = np.asarray(hidden, dtype=np.float32)
    ids = np.asarray(batch_ids).astype(np.int64)
    assert hidden.shape == (N, D), hidden.shape

    counts = np.bincount(ids, minlength=C).astype(np.float64)
    sums, sumsq, _ = _device_stats(hidden, ids)
    total = _pairwise_loss(counts, sums, sumsq, int(np.asarray(d)))
    return np.array(total, dtype=np.float32)
